# revision 1
# baseline (speedup 1.0000x reference)
"""Disentangled self-attention (DeBERTa-style) on 8 TRN2 NeuronCores.

Problem: B=4, L=256, D=512, H=8, R=64 rel-pos buckets, DK=64.
Sharding: core c handles batch b=c//2, query rows l0=128*(c%2) .. l0+128.
No cross-core communication (output rows are disjoint).

Device dataflow per core (all matmuls accumulate f32 in PSUM):
  - constants arrive as 3 packed blobs (128-part / 64-part / 1-part) to
    minimize DMA-issue serialization at kernel start
  - projections in fp16, feature-major q/k ([o,l]) and token-major v ([m,o]),
    bias folded via an appended ones-row / bias-row (aug) on the host
  - scores psum A[l,h,m] = additive key mask (rank-1, seeds each bank)
      + q.k per head + position->content term (one matmul per key m-pair,
      bank-crossing strided out AP accumulates straight into A)
      + transposed-accumulated content->position term (t1)
  - gathers use host-built one-hot matrices O1[r,l,m] / O2[r,m,l] (fp8e3m4,
    exact for 0/1) as weights against fp16 c2p/p2c moving operands
  - softmax per head-pair on DVE/ACT with a constant exp-shift (logits are
    bounded, softmax is shift-invariant), pipelined under the PE transposes;
    pT transposes lag one pair so the PE FIFO never stalls on a softmax
  - ctx via head-pair psums [128, 2*128]: v-seed matmul + one fp8-weight
    [128x128]@[128x2] matmul per (query row, head pair, key chunk); rel_v
    streams as fp8e3m4 weights (halves HBM traffic), p stays fp16
  - output projection from the diagonal-extracted ctxT, f32 result to DRAM
"""

import sys

for _p in ("/opt/trn_rl_repo", "/root/.axon_site/_ro/trn_rl_repo"):
    if _p not in sys.path:
        sys.path.append(_p)

import numpy as np
import ml_dtypes

import concourse.bass as bass
import concourse.tile as tile
from concourse import bacc, mybir
from concourse.bass_utils import run_bass_kernel_spmd
from concourse.masks import make_identity

B, L, D, H = 4, 256, 512, 8
R = 64
DK = D // H
LH = 128                      # query rows per core
NCORES = 8
SCALE = float(1.0 / (3.0 * np.sqrt(np.float32(DK))))
MASKVAL = -60000.0            # exp() underflows identically to the ref's -1e9
RVG = 8                       # query rows per rel_v DMA group
NG = LH // RVG

F16 = mybir.dt.float16
F32 = mybir.dt.float32
E3 = mybir.dt.float8e3
EXP = mybir.ActivationFunctionType.Exp
AX = mybir.AxisListType.X

NP_E3 = ml_dtypes.float8_e3m4

# blobA1: q-projection consts land first (small -> projections start early)
A_WQ, A_XQ = 0, 2048
A1_END = 2560
# blobA2: the rest of the 128-partition consts
A_WK, A_WV, A_WO = 0, 2048, 4096
A_XK, A_XV = 6144, 7168
A2_END = 8192
# blobB column offsets (64-partition consts, fp16): rel_k, rel_q, then
# per-head bias columns for q/k (bias applied via ACT copy, not matmul)
B_RK, B_RQ, B_BQ, B_BK = 0, 512, 1024, 1032
B_END = 1040
# blobC column offsets (1-partition consts, fp16)
C_WVB, C_XVB = 0, 512
C_MASK, C_ONES = 768, 2816
C_END = 2944


def build_nc(phase=99, sub="all"):
    nc = bacc.Bacc(None, target_bir_lowering=False)

    # ---- DRAM I/O (per-core shard shapes) ----
    d_bA1 = nc.dram_tensor("blobA1", [128, A1_END], F16, kind="ExternalInput")
    d_bA2 = nc.dram_tensor("blobA2", [128, A2_END], F16, kind="ExternalInput")
    d_bB = nc.dram_tensor("blobB", [DK, B_END], F16, kind="ExternalInput")
    d_bC = nc.dram_tensor("blobC", [1, C_END], F16, kind="ExternalInput")
    # stacked-pair one-hots: row r + 64*j holds pair-member j (l=2p+j / m=2p+j)
    d_O1 = nc.dram_tensor("O1", [128, LH // 2, L], E3, kind="ExternalInput")
    d_O2 = nc.dram_tensor("O2", [128, L // 2, LH], E3, kind="ExternalInput")
    # rel_v fp8, grouped RVG rows per DMA: [g, m0, j, c, f]
    d_rv = nc.dram_tensor("rv", [NG, 128, RVG, 2, D], E3, kind="ExternalInput")
    d_out = nc.dram_tensor("out", [LH, D], F32, kind="ExternalOutput")

    with tile.TileContext(nc) as tc:
        with (
            tc.tile_pool(name="consts", bufs=1) as consts,
            tc.tile_pool(name="work", bufs=1) as work,
            tc.tile_pool(name="sm", bufs=2) as smp,
            tc.tile_pool(name="rvp", bufs=12) as rvp,
        ):
            dbg_ap = None

            # everything rides the sync HWDGE ring, FIFO in priority order:
            # q-proj blob -> small blobs -> rest -> one-hots -> rel_v -> out
            bA1 = consts.tile([128, A1_END], F16, tag="bA1", name="bA1")
            nc.sync.dma_start(out=bA1[:], in_=d_bA1[:, :])
            bB = consts.tile([DK, B_END], F16, tag="bB", name="bB")
            nc.sync.dma_start(out=bB[:], in_=d_bB[:, :])
            bC = consts.tile([1, C_END], F16, tag="bC", name="bC")
            nc.sync.dma_start(out=bC[:], in_=d_bC[:, :])
            bA2 = consts.tile([128, A2_END], F16, tag="bA2", name="bA2")
            nc.sync.dma_start(out=bA2[:], in_=d_bA2[:, :])
            o1t = consts.tile([128, LH // 2, L], E3, tag="o1t")
            nc.sync.dma_start(out=o1t[:], in_=d_O1[:, :, :])
            o2t = consts.tile([128, L // 2, LH], E3, tag="o2t")
            nc.sync.dma_start(out=o2t[:], in_=d_O2[:, :, :])

            id16 = consts.tile([128, 128], F16, tag="id16")
            make_identity(nc, id16[:])
            id32 = consts.tile([128, 128], F32, tag="id32")
            make_identity(nc, id32[:])

            # q/k bias columns as f32 for ACT-copy bias application
            bqk32 = consts.tile([DK, 16], F32, tag="bqk32", name="bqk32")
            nc.vector.tensor_copy(bqk32[:], bB[:, B_BQ:B_BK + 8])
            # constant exp-shift for the rowmax-free softmax
            negb = consts.tile([128, 1], F32, tag="negb", name="negb")
            nc.vector.memset(negb[:], -3.0)

            # PE warmup: dummy matmuls fill the blob-DMA wait and hold the
            # HAM clock gate open so projections run at full clock
            with tc.tile_pool(name="pwarm", bufs=1, space="PSUM") as pwarm:
                wps = pwarm.tile([128, 128], F32, tag="warm", name="warm")
                for _ in range(55):
                    nc.tensor.matmul(wps[:], id16[:], id16[:], start=True, stop=True)

            if phase == 0:
                dbg_ap = bA1[:, A_XQ:A_XQ + 128]

            # ---------- projections ----------
            if phase >= 1:
                qf2 = work.tile([DK, H, LH], F16, tag="qf2", name="qf2")
                kf2 = work.tile([DK, H, L], F16, tag="kf2", name="kf2")
                vp = [work.tile([128, D], F16, tag=f"vp{i}", name=f"vp{i}") for i in range(2)]

                # j-split copies of q/k ([dk, h, member, half-rows]) feed the
                # c2p/p2c matmuls; block-diag pair rhs assembled directly from
                # col-group-tiled psum (odd member lands on partitions 64-127)
                qf2j = work.tile([DK, H, 2, LH // 2], F16, tag="qf2j", name="qf2j")
                kf2j = work.tile([DK, H, 2, L // 2], F16, tag="kf2j", name="kf2j")
                c2p2 = work.tile([128, 16, LH // 2], F16, tag="c2p2", name="c2p2")
                p2c2 = work.tile([128, 16, L // 2], F16, tag="p2c2", name="p2c2")
                nc.vector.memset(c2p2[0:64, 8:16, :], 0.0)
                nc.vector.memset(c2p2[64:128, 0:8, :], 0.0)
                nc.vector.memset(p2c2[0:64, 8:16, :], 0.0)
                nc.vector.memset(p2c2[64:128, 0:8, :], 0.0)

                with tc.tile_pool(name="pproj", bufs=3, space="PSUM") as pproj:
                    for h in range(H):
                        o = A_WQ + h * 64
                        ps = pproj.tile([DK, LH], F32, tag="pp", name="pp")
                        for kc in range(4):
                            nc.tensor.matmul(ps[:], bA1[:, o + kc * 512:o + kc * 512 + 64],
                                             bA1[:, A_XQ + kc * 128:A_XQ + (kc + 1) * 128],
                                             start=(kc == 0), stop=(kc == 3))
                        nc.scalar.activation(qf2[:, h, :], ps[:],
                                             mybir.ActivationFunctionType.Identity,
                                             bias=bqk32[:, h:h + 1])
                        nc.scalar.activation(qf2j[:, h, :, :],
                                             ps[:].rearrange("d (p j) -> d j p", j=2),
                                             mybir.ActivationFunctionType.Identity,
                                             bias=bqk32[:, h:h + 1])
                    for h in range(H):
                        o = A_WK + h * 64
                        ps = pproj.tile([DK, L], F32, tag="pp", name="pp")
                        for kc in range(4):
                            nc.tensor.matmul(ps[:], bA2[:, o + kc * 512:o + kc * 512 + 64],
                                             bA2[:, A_XK + kc * 256:A_XK + (kc + 1) * 256],
                                             start=(kc == 0), stop=(kc == 3))
                        nc.scalar.activation(kf2[:, h, :], ps[:],
                                             mybir.ActivationFunctionType.Identity,
                                             bias=bqk32[:, 8 + h:8 + h + 1])
                        nc.scalar.activation(kf2j[:, h, :, :],
                                             ps[:].rearrange("d (p j) -> d j p", j=2),
                                             mybir.ActivationFunctionType.Identity,
                                             bias=bqk32[:, 8 + h:8 + h + 1])
                    for mc in range(2):
                        ps = pproj.tile([128, D], F32, tag="pp", name="pp")
                        for kc in range(4):
                            nc.tensor.matmul(
                                ps[:],
                                bA2[:, A_XV + kc * 256 + mc * 128:A_XV + kc * 256 + mc * 128 + 128],
                                bA2[:, A_WV + kc * 512:A_WV + (kc + 1) * 512],
                                start=(kc == 0), stop=False)
                        nc.tensor.matmul(ps[:], bC[:, C_XVB + mc * 128:C_XVB + mc * 128 + 128],
                                         bC[:, C_WVB:C_WVB + 512], start=False, stop=True)
                        nc.vector.tensor_copy(vp[mc][:], ps[:])

                    # c2p/p2c per (head, pair-member); odd member col-tiled to
                    # psum partitions 64-127 so the block-diag rhs assembles
                    # with partition-preserving DVE copies (no shift DMA)
                    for h in range(H):
                        psc = pproj.tile([128, LH // 2], F32, tag="pc", name="pc")
                        nc.tensor.matmul(psc[0:64, :], bB[:, B_RK + h * 64:B_RK + (h + 1) * 64],
                                         qf2j[:, h, 0, :], start=True, stop=True)
                        nc.tensor.matmul(psc[64:128, :], bB[:, B_RK + h * 64:B_RK + (h + 1) * 64],
                                         qf2j[:, h, 1, :], start=True, stop=True,
                                         tile_position=(0, 64))
                        nc.vector.tensor_copy(c2p2[0:64, h, :], psc[0:64, :])
                        nc.vector.tensor_copy(c2p2[64:128, 8 + h, :], psc[64:128, :])
                        ps2 = pproj.tile([128, L // 2], F32, tag="pc", name="pc")
                        nc.tensor.matmul(ps2[0:64, :], bB[:, B_RQ + h * 64:B_RQ + (h + 1) * 64],
                                         kf2j[:, h, 0, :], start=True, stop=True)
                        nc.tensor.matmul(ps2[64:128, :], bB[:, B_RQ + h * 64:B_RQ + (h + 1) * 64],
                                         kf2j[:, h, 1, :], start=True, stop=True,
                                         tile_position=(0, 64))
                        nc.vector.tensor_copy(p2c2[0:64, h, :], ps2[0:64, :])
                        nc.vector.tensor_copy(p2c2[64:128, 8 + h, :], ps2[64:128, :])

                if phase == 1:
                    dbg_ap = vp[0][:]

            # ---------- scores + softmax ----------
            _lv = {"qk": 0, "t1": 1, "tr": 2, "B": 3, "sm": 4, "all": 9}[sub]
            if phase >= 2:
                with tc.tile_pool(name="pscA", bufs=1, space="PSUM") as pscA:
                    A = pscA.tile([128, H, L], F32, tag="A", name="A")    # 4 banks
                    # mask seeds each bank (start=True covers 2 heads)
                    for h2 in range(0, H, 2):
                        nc.tensor.matmul(A[:, h2:h2 + 2, :], bC[:, C_ONES:C_ONES + LH],
                                         bC[:, C_MASK + h2 * L:C_MASK + (h2 + 2) * L],
                                         start=True, stop=False)
                    for h in range(H):
                        nc.tensor.matmul(A[:, h, :], qf2[:, h, :], kf2[:, h, :],
                                         start=False, stop=False)

                    # term2 accumulates DIRECTLY into A via a strided
                    # bank-crossing out AP: col (h, m=2p+j) from matmul cols
                    # (j, h) -- no B psum tile, no B_sb copy, no softmax add
                    for p in range(L // 2 if _lv >= 3 else 0):
                        nc.tensor.matmul(
                            A[:, :, 2 * p:2 * p + 2].rearrange("l h j -> l j h"),
                            o2t[:, p, :], p2c2[:, :, p],
                            start=False, stop=False, skip_group_check=True)

                    t1s = [work.tile([128, LH, H], F32, tag=f"t1s{mc}", name=f"t1s{mc}")
                           for mc in range(2)]
                    with tc.tile_pool(name="pscB", bufs=2, space="PSUM") as pscB:
                        # t1: psum t1T[m, l, h] per m-chunk -> sb -> PE-transpose into A
                        for mc in range(2 if _lv >= 1 else 0):
                            t1 = pscB.tile([128, LH, H], F32, tag="big", name="big")
                            for p in range(LH // 2):
                                nc.tensor.matmul(t1[:, 2 * p:2 * p + 2, :],
                                                 o1t[:, p, mc * 128:(mc + 1) * 128],
                                                 c2p2[:, :, p],
                                                 start=(p % 32 == 0), stop=(p % 32 == 31))
                            nc.vector.tensor_copy(t1s[mc][:], t1[:])

                    # transpose t1 into A, closing one bank per head-pair, then
                    # run that pair's softmax + p-transposes immediately so the
                    # DVE/ACT chain pipelines under the next pair's PE work
                    p16 = work.tile([128, H, L], F16, tag="p16", name="p16")
                    sums = work.tile([128, H], F32, tag="sums", name="sums")
                    recs = work.tile([128, H], F32, tag="recs", name="recs")
                    pT = [work.tile([128, H, LH], F16, tag=f"pT{c}", name=f"pT{c}")
                          for c in range(2)]
                    def tr_pair(hq):
                        for h in (2 * hq, 2 * hq + 1):
                            for mc in range(2 if _lv >= 2 else 0):
                                nc.tensor.matmul(A[:, h, mc * 128:(mc + 1) * 128],
                                                 t1s[mc][:, :, h], id32[:],
                                                 is_transpose=True, start=False,
                                                 stop=(mc == 1 and h % 2 == 1))

                    def sm_pair(hq):
                        # logits are bounded (|s| < 2.6 by construction), so a
                        # constant exp-shift replaces the rowmax pass; softmax
                        # is shift-invariant so the result is exact
                        for h in ((2 * hq, 2 * hq + 1) if _lv >= 4 else ()):
                            e = smp.tile([128, L], F32, tag="e", name="e")
                            nc.scalar.activation(e[:], A[:, h, :], EXP, bias=negb[:],
                                                 scale=1.0, accum_out=sums[:, h:h + 1])
                            nc.vector.reciprocal(recs[:, h:h + 1], sums[:, h:h + 1])
                            nc.vector.tensor_scalar_mul(p16[:, h, :], e[:], recs[:, h:h + 1])

                    def pt_pair(hq, ppt):
                        for h in ((2 * hq, 2 * hq + 1) if phase >= 4 and _lv >= 4 else ()):
                            for c in range(2):
                                pps = ppt.tile([128, 128], F16, tag="pt", name="pt")
                                nc.tensor.matmul(pps[:], p16[:, h, c * 128:(c + 1) * 128],
                                                 id16[:], is_transpose=True)
                                nc.vector.tensor_copy(pT[c][:, h, :], pps[:])

                    # pT transposes lag one pair behind the tr matmuls so the
                    # PE FIFO never stalls waiting on a pair's softmax chain
                    with tc.tile_pool(name="ppt", bufs=2, space="PSUM") as ppt:
                        tr_pair(0); sm_pair(0)
                        tr_pair(1); sm_pair(1)
                        pt_pair(0, ppt)
                        tr_pair(2); sm_pair(2)
                        pt_pair(1, ppt)
                        tr_pair(3); sm_pair(3)
                        pt_pair(2, ppt)
                        pt_pair(3, ppt)

                if phase == 2:
                    dbg_ap = {0: A[:, 0, :], 1: t1s[0][:, :, 0], 2: A[:, 0, :],
                              3: p16[:, 0, :], 4: p16[:, 0, :], 9: p16[:, 0, :]}[_lv]
                if phase == 3:
                    dbg_ap = p16[:, 0, :]

            # ---------- ctx + output projection ----------
            if phase >= 4:
                with tc.tile_pool(name="pctx", bufs=1, space="PSUM") as pctx:
                    cp = [pctx.tile([128, 2 * LH], F32, tag=f"cp{hp}", name=f"cp{hp}")
                          for hp in range(4)]
                    nrv = NG if phase >= 5 else 0
                    for hp in range(4):
                        for c in range(2):
                            rhs = pT[c][:, 2 * hp:2 * hp + 2, :].rearrange("p hh l -> p l hh")
                            nc.tensor.matmul(cp[hp][:], vp[c][:, hp * 128:(hp + 1) * 128],
                                             rhs, start=(c == 0),
                                             stop=(c == 1 and nrv == 0))
                    for g in range(nrv):
                        rvt = rvp.tile([128, RVG, 2, D], E3, tag="rv", name="rv")
                        nc.sync.dma_start(out=rvt[:], in_=d_rv[g])
                        for j in range(RVG):
                            l = RVG * g + j
                            for hp in range(4):
                                for c in range(2):
                                    nc.tensor.matmul(
                                        cp[hp][:, 2 * l:2 * l + 2],
                                        rvt[:, j, c, hp * 128:(hp + 1) * 128],
                                        pT[c][:, 2 * hp:2 * hp + 2, l:l + 1],
                                        start=False, stop=(c == 1 and l == LH - 1))

                    ctxT = [work.tile([128, LH], F16, tag=f"ctxT{hp}", name=f"ctxT{hp}")
                            for hp in range(4)]
                    for hp in range(4):
                        nc.vector.tensor_copy(
                            ctxT[hp][0:64, :],
                            cp[hp][0:64, :].rearrange("p (l hh) -> p hh l", hh=2)[:, 0, :])
                        nc.vector.tensor_copy(
                            ctxT[hp][64:128, :],
                            cp[hp][64:128, :].rearrange("p (l hh) -> p hh l", hh=2)[:, 1, :])
                    ops = pctx.tile([128, D], F32, tag="ops", name="ops")
                    for hp in range(4):
                        nc.tensor.matmul(ops[:], ctxT[hp][:],
                                         bA2[:, A_WO + hp * 512:A_WO + (hp + 1) * 512],
                                         start=(hp == 0), stop=(hp == 3))
                    out_sb = work.tile([128, D], F32, tag="out_sb", name="out_sb")
                    nc.vector.tensor_copy(out_sb[:], ops[:])
                    nc.sync.dma_start(out=d_out[:, :], in_=out_sb[:])

            if phase < 4:
                dbg = work.tile([128, D], F32, tag="dbg", name="dbg")
                nc.vector.memset(dbg[:], 0.0)
                n = min(int(np.prod(dbg_ap.shape[1:])), D)
                nc.vector.tensor_copy(dbg[:dbg_ap.shape[0], 0:n], dbg_ap[:, 0:n])
                nc.sync.dma_start(out=d_out[:, :], in_=dbg[:])

    nc.finalize()
    return nc


_NC_CACHE = None


def _get_nc():
    global _NC_CACHE
    if _NC_CACHE is None:
        import os
        _NC_CACHE = build_nc(int(os.environ.get("KPHASE", "99")),
                             os.environ.get("KSUB", "all"))
    return _NC_CACHE


def _c4(x):
    # [512, N] -> [128, 4*N] with element (p, c*N+f) = x[c*128+p, f]
    n = x.shape[1]
    return x.reshape(4, 128, n).transpose(1, 0, 2).reshape(128, 4 * n)


def host_prep(inputs, c):
    f16, f32 = np.float16, np.float32
    b, lh = c // 2, c % 2
    l0 = lh * LH
    q = np.asarray(inputs["query"][b], f32)
    k = np.asarray(inputs["key"][b], f32)
    v = np.asarray(inputs["value"][b], f32)
    mask = np.asarray(inputs["mask"][b])
    rp = np.asarray(inputs["rel_pos"][b], np.int64)
    rv = np.asarray(inputs["rel_v"][b], f32)

    WqT = np.asarray(inputs["Wq"], f32).T
    WkT = np.asarray(inputs["Wk"], f32).T
    WvT = np.asarray(inputs["Wv"], f32).T
    WoT = np.asarray(inputs["Wo"], f32).T
    qT = q[l0:l0 + LH].T * SCALE                        # [512, LH]
    kT = k.T                                            # [512, L]
    vT = v.T

    blobA1 = np.concatenate([_c4(WqT), _c4(qT)], axis=1).astype(f16)
    assert blobA1.shape == (128, A1_END)
    blobA2 = np.concatenate(
        [_c4(WkT), _c4(WvT), _c4(WoT), _c4(kT), _c4(vT)], axis=1).astype(f16)
    assert blobA2.shape == (128, A2_END)

    blobB = np.concatenate(
        [np.asarray(inputs["rel_k"], f32).transpose(2, 0, 1).reshape(DK, 512),
         np.asarray(inputs["rel_q"], f32).transpose(2, 0, 1).reshape(DK, 512) * SCALE,
         np.asarray(inputs["bq"], f32).reshape(H, DK).T * SCALE,
         np.asarray(inputs["bk"], f32).reshape(H, DK).T],
        axis=1).astype(f16)
    assert blobB.shape == (DK, B_END)

    mrow = np.where(mask, np.float32(MASKVAL), np.float32(0.0))
    blobC = np.concatenate(
        [np.asarray(inputs["bv"], f32),
         np.ones(L, f32),
         np.tile(mrow, H),
         np.ones(LH, f32)])[None, :].astype(f16)
    assert blobC.shape == (1, C_END)

    d = {"blobA1": blobA1, "blobA2": blobA2, "blobB": blobB, "blobC": blobC}
    rp_c = rp[l0:l0 + LH]
    eye = np.eye(R, dtype=f32)
    O1 = eye[:, rp_c]                                   # [R, LH, L]
    O2 = eye[:, rp_c.T]                                 # [R, L, LH]
    d["O1"] = np.ascontiguousarray(
        O1.reshape(R, LH // 2, 2, L).transpose(2, 0, 1, 3)
        .reshape(128, LH // 2, L)).astype(NP_E3)
    d["O2"] = np.ascontiguousarray(
        O2.reshape(R, L // 2, 2, LH).transpose(2, 0, 1, 3)
        .reshape(128, L // 2, LH)).astype(NP_E3)
    # rel_v: [g, m0, j, c, f] where l = RVG*g + j, m = c*128 + m0
    rv_c = rv[l0:l0 + LH]                               # [LH, L, D]
    d["rv"] = np.ascontiguousarray(
        rv_c.reshape(NG, RVG, 2, 128, D).transpose(0, 3, 1, 2, 4)).astype(NP_E3)
    return d


def kernel(**inputs) -> np.ndarray:
    nc = _get_nc()
    in_maps = [host_prep(inputs, c) for c in range(NCORES)]
    res = run_bass_kernel_spmd(nc, in_maps, core_ids=list(range(NCORES)))
    out = np.zeros((B, L, D), np.float32)
    for c in range(NCORES):
        b, lh = c // 2, c % 2
        out[b, lh * LH:(lh + 1) * LH] = res.results[c]["out"]
    out += np.asarray(inputs["bo"], np.float32)[None, None, :]
    return out

